# revision 24
# baseline (speedup 1.0000x reference)
"""GAT (2-layer, 8-head) Trainium2 Bass kernel, SPMD over 8 NeuronCores. v2.

Strategy (dst-node-parallel, fp8 tables, host-precomputed onehots):
  - Host: sort edges by dst; each core owns a 1250-node dst range (10 tiles of
    128). Edges per tile padded to 256-multiples (fp8 DoubleRow chunks).
    Host precomputes: fused projection weights, per-slot onehot matrices
    oh[p,hc,j] (edge->dst-local) and their transposes ohT[j,hc,p] in fp8, and
    wrapped int16 gather indices.
  - Table rows are 1280B fp8: [h fp8 1024B | sj bf16 16B | si fp8 8B | pad].
  - Layer-1 projection is SHARDED: each core projects only its 1250 nodes
    (bf16 matmuls for precision), then AllGather assembles the full fp8 table.
  - Edge phase per tile: dma_gather rows by src; si of the 128 owned dst nodes
    comes from a tiny own-row gather + per-halfchunk onehot-transpose matmuls
    (no per-edge si gather). Logits -> Lrelu -> Exp(-ln8) on ACT. Messages
    alpha*h via ONE broadcast tensor_tensor per 256-edge chunk (fp8 out).
    Aggregation via fp8 DoubleRow onehot-matmuls into PSUM (denominator =
    extra matmul columns). Head-mean via mult+reduce, ELU -> emb tile.
  - AllGather emb1 (bf16), replicated bf16 layer-2 projection, second edge
    phase -> per-core partial graph-sum [1, 128].
  - Host: sum partials, LayerNorm + MLP head -> [16].
"""

import numpy as np
import ml_dtypes

BF16 = ml_dtypes.bfloat16

N_NODES = 10000
N_EDGES = 160000
N_FEAT = 512
HEADS = 8
HID = 128
OUT = 16

N_CORES = 8
P = 128

nloc = N_NODES // N_CORES          # 1250
NPADC = 1280                       # per-core padded node rows
T = NPADC // P                     # 10 tiles per core
HD = HEADS * HID                   # 1024
PCOLS = HD + 16                    # proj psum cols: [h | sj 8 | si 8]
ROWB = 1280                        # table row bytes (fp8 dtype)
LN8 = float(np.log(8.0))


def _wrap_idx(idx: np.ndarray) -> np.ndarray:
    """[n] -> [128, n//16] wrapped int16 layout for dma_gather."""
    n = idx.shape[0]
    assert n % 16 == 0
    w = idx.astype(np.int16).reshape(n // 16, 16).T
    return np.tile(w, (8, 1))


def host_prep(node_features, edge_src, edge_dst, W1, a1, W2, a2):
    F8 = ml_dtypes.float8_e4m3

    order = np.argsort(edge_dst, kind="stable")
    src_s = edge_src[order].astype(np.int64)
    dst_s = edge_dst[order].astype(np.int64)

    # per (core, tile) edge counts -> common 256-chunk counts per tile pos
    cnt = np.zeros((N_CORES, T), dtype=np.int64)
    bounds = {}
    for c in range(N_CORES):
        base = c * nloc
        for t in range(T):
            n0 = base + t * P
            n1 = min(base + (t + 1) * P, (c + 1) * nloc)
            lo = np.searchsorted(dst_s, n0, side="left")
            hi = np.searchsorted(dst_s, n1, side="left")
            cnt[c, t] = hi - lo
            bounds[(c, t)] = (lo, hi)
    HC = [int(-(-cnt[:, t].max() // 128)) for t in range(T)]  # 128-halfchunks
    tot_hc = sum(HC)
    tot_slots = tot_hc * P

    # fused projection matrices [K, 1040] = [h | sj | si]
    def wcat(W, a, K):
        w = np.transpose(W, (2, 0, 1)).reshape(K, HD)
        si = np.einsum("hdf,hd->fh", W, a[:, :HID])   # dst-side
        sj = np.einsum("hdf,hd->fh", W, a[:, HID:])   # src-side
        return np.concatenate([w, sj, si], axis=1).astype(BF16)

    w1c = wcat(W1, a1, N_FEAT)   # [512, 1040]
    w2c = wcat(W2, a2, HID)      # [128, 1040]
    # sbuf layouts: w1s[p, k, col] = w1c[k*128+p, col]
    w1s = np.ascontiguousarray(
        w1c.reshape(4, P, PCOLS).transpose(1, 0, 2))      # [128, 4, 1040]
    w2s = np.ascontiguousarray(w2c.reshape(1, P, PCOLS).transpose(1, 0, 2))

    xbf = node_features.astype(BF16)

    in_maps = []
    for c in range(N_CORES):
        base = c * nloc
        src_pad = np.zeros(tot_slots, dtype=np.int64)
        dstl = np.full(tot_slots, -1, dtype=np.int64)  # -1 -> pad slot
        off = 0
        for t in range(T):
            lo, hi = bounds[(c, t)]
            k = hi - lo
            src_pad[off:off + k] = src_s[lo:hi]
            dstl[off:off + k] = dst_s[lo:hi] - (base + t * P)
            off += HC[t] * P
        assert off == tot_slots

        # onehots fp8: oh[p, hc, j], ohT[j, hc, p]
        sl = src_pad.reshape(tot_hc, P)       # slot (hc, p)
        dl = dstl.reshape(tot_hc, P)
        jj = np.arange(P)
        oh = (dl[:, :, None] == jj[None, None, :])        # [hc, p, j]
        oh8 = np.ascontiguousarray(
            oh.transpose(1, 0, 2).astype(F8).reshape(P, tot_hc * P))
        ohT8 = np.ascontiguousarray(
            oh.transpose(2, 0, 1).astype(F8).reshape(P, tot_hc * P))

        srcI = np.concatenate(
            [_wrap_idx(sl[sum(HC[:t]):sum(HC[:t + 1])].reshape(-1))
             for t in range(T)], axis=1)                  # [128, tot_slots/16]
        own = np.minimum(np.arange(NPADC), nloc - 1)      # local row ids
        own16 = _wrap_idx(own)                            # [128, 80]

        # per-core x slice, transposed: xT[p, k, n] = x[base+n, k*128+p]
        xsl = np.zeros((NPADC, N_FEAT), dtype=BF16)
        xsl[:nloc] = xbf[base:base + nloc]
        xT = np.ascontiguousarray(
            xsl.reshape(NPADC, 4, P).transpose(2, 1, 0))  # [128, 4, 1280]

        in_maps.append({
            "xT": xT, "w1s": w1s, "w2s": w2s,
            "srcI": np.ascontiguousarray(srcI), "own16": own16,
            "oh8": oh8, "ohT8": ohT8,
        })

    meta = {"HC": HC, "tot_hc": tot_hc}
    return in_maps, meta


def build_program(meta, debug=False, stages=5, iters=1):
    import concourse.bacc as bacc
    import concourse.mybir as mybir
    import concourse.tile as tile
    from concourse.library_config import mlp

    dt = mybir.dt
    Alu = mybir.AluOpType
    Act = mybir.ActivationFunctionType
    DR = mybir.MatmulPerfMode.DoubleRow

    HC = meta["HC"]
    tot_hc = meta["tot_hc"]

    nc = bacc.Bacc("TRN2", num_devices=N_CORES, num_swdge_queues=2,
                   dynamic_dma_scratch_size=49152)

    xT_d = nc.dram_tensor("xT", [P, 4, NPADC], dt.bfloat16, kind="ExternalInput")
    w1_d = nc.dram_tensor("w1s", [P, 4, PCOLS], dt.bfloat16, kind="ExternalInput")
    w2_d = nc.dram_tensor("w2s", [P, 1, PCOLS], dt.bfloat16, kind="ExternalInput")
    srcI_d = nc.dram_tensor("srcI", [P, tot_hc * 8], dt.int16, kind="ExternalInput")
    own_d = nc.dram_tensor("own16", [P, NPADC // 16], dt.int16, kind="ExternalInput")
    oh_d = nc.dram_tensor("oh8", [P, tot_hc * P], dt.float8e4, kind="ExternalInput")
    ohT_d = nc.dram_tensor("ohT8", [P, tot_hc * P], dt.float8e4, kind="ExternalInput")

    out_vec = nc.dram_tensor("out_vec", [1, HID], dt.float32, kind="ExternalOutput")
    dbg = {}
    if debug:
        dbg["emb1"] = nc.dram_tensor("dbg_emb1", [N_NODES, HID], dt.float32,
                                     kind="ExternalOutput")
        dbg["tab1"] = nc.dram_tensor("dbg_tab1", [P, ROWB], dt.float32,
                                     kind="ExternalOutput")

    t1loc = nc.dram_tensor("t1loc", [NPADC, ROWB], dt.float8e4)
    table1 = nc.dram_tensor("table1", [N_NODES, ROWB], dt.float8e4,
                            addr_space="Shared")
    t2loc = nc.dram_tensor("t2loc", [NPADC, ROWB], dt.float8e4)
    table2 = nc.dram_tensor("table2", [N_NODES, ROWB], dt.float8e4,
                            addr_space="Shared")
    emb_loc = nc.dram_tensor("emb_loc", [NPADC, HID], dt.bfloat16)

    with tile.TileContext(nc) as tc:
        with (
            tc.tile_pool(name="const", bufs=1) as cpool,
            tc.tile_pool(name="xload", bufs=2) as xpool,
            tc.tile_pool(name="single", bufs=1) as spool,
            tc.tile_pool(name="work", bufs=2) as wpool,
            tc.tile_pool(name="chunk", bufs=3) as kpool,
            tc.tile_pool(name="psum", bufs=2, space="PSUM") as pspool,
            tc.tile_pool(name="psi", bufs=1, space="PSUM") as psip,
            tc.tile_pool(name="psg", bufs=1, space="PSUM") as psg,
        ):
            nc.gpsimd.load_library(mlp)

            ones_col = cpool.tile([P, 1], dt.bfloat16)
            nc.gpsimd.memset(ones_col[:], 1.0)
            negln8 = cpool.tile([P, 1], dt.float32)
            nc.gpsimd.memset(negln8[:], -LN8)
            zeros = cpool.tile([P, HID], dt.float32)
            nc.gpsimd.memset(zeros[:], 0.0)

            w1s = cpool.tile([P, 4, PCOLS], dt.bfloat16)
            nc.sync.dma_start(out=w1s[:], in_=w1_d[:])
            w2s = cpool.tile([P, 1, PCOLS], dt.bfloat16)
            nc.sync.dma_start(out=w2s[:], in_=w2_d[:])
            srcI = cpool.tile([P, tot_hc * 8], dt.int16)
            nc.sync.dma_start(out=srcI[:], in_=srcI_d[:])
            ownI = cpool.tile([P, NPADC // 16], dt.int16)
            nc.sync.dma_start(out=ownI[:], in_=own_d[:])
            ohsb = cpool.tile([P, tot_hc, P], dt.float8e4)
            nc.scalar.dma_start(out=ohsb[:],
                                in_=oh_d[:].rearrange("p (c j) -> p c j", j=P))
            ohtsb = cpool.tile([P, tot_hc, P], dt.float8e4)
            nc.scalar.dma_start(out=ohtsb[:],
                                in_=ohT_d[:].rearrange("p (c j) -> p c j", j=P))

            # ---------------- row cast helper ----------------
            def row_cast(ps, psc, row, s):
                """PSUM h[*,1024]+scores[*,16] f32 -> fp8 row at row[:,s,:]."""
                nc.scalar.activation(row[:, s, 0:512], ps[:, 0:512], Act.Copy)
                nc.vector.tensor_copy(row[:, s, 512:HD], ps[:, 512:HD])
                nc.vector.tensor_copy(
                    row[:, s, HD:HD + 16].bitcast(dt.bfloat16), psc[:, 0:8])
                nc.vector.tensor_copy(row[:, s, HD + 16:HD + 24], psc[:, 8:16])

            # ---------------- layer-1 sharded projection ----------------
            def proj1():
                xsb = spool.tile([P, 4, NPADC], dt.bfloat16, tag="xsb")
                nc.sync.dma_start(out=xsb[:], in_=xT_d[:])
                row = spool.tile([P, T, ROWB], dt.float8e4, tag="rowA")
                for s in range(T):
                    ps = pspool.tile([P, HD], dt.float32, tag="ps")
                    psc = psip.tile([P, 16], dt.float32, tag="psc")
                    for k in range(4):
                        lhsT = xsb[:, k, s * P:(s + 1) * P]
                        st, sp = (k == 0), (k == 3)
                        nc.tensor.matmul(ps[:, 0:512], lhsT=lhsT,
                                         rhs=w1s[:, k, 0:512], start=st, stop=sp)
                        nc.tensor.matmul(ps[:, 512:1024], lhsT=lhsT,
                                         rhs=w1s[:, k, 512:1024], start=st, stop=sp)
                        nc.tensor.matmul(psc[:, 0:16], lhsT=lhsT,
                                         rhs=w1s[:, k, 1024:PCOLS], start=st, stop=sp)
                    row_cast(ps, psc, row, s)
                nc.sync.dma_start(
                    out=t1loc[:].rearrange("(a p) c -> p a c", p=P),
                    in_=row[:])
                psis1 = edges_prep(t1loc)
                nc.gpsimd.collective_compute(
                    "AllGather", mybir.AluOpType.bypass,
                    ins=[t1loc[0:nloc, :]], outs=[table1[:]],
                    replica_groups=[list(range(N_CORES))])
                return psis1

            # ---------------- layer-2 sharded projection ----------------
            def proj2():
                row = spool.tile([P, T, ROWB], dt.float8e4, tag="rowA",
                                 name="row2")
                for b in range(3):
                    r0 = b * 512
                    rn = min(512, NPADC - r0)         # 512, 512, 256
                    eT = xpool.tile([P, 512], dt.bfloat16, tag="eT")
                    eng = nc.sync if b % 2 == 0 else nc.scalar
                    eng.dma_start_transpose(eT[:, :rn], emb_loc[r0:r0 + rn, :])
                    for s in range(-(-rn // P)):
                        nn = min(P, rn - s * P)
                        ps = pspool.tile([P, HD], dt.float32, tag="ps")
                        psc = psip.tile([P, 16], dt.float32, tag="psc")
                        lhsT = eT[:, s * P:s * P + nn]
                        nc.tensor.matmul(ps[:nn, 0:512], lhsT=lhsT,
                                         rhs=w2s[:, 0, 0:512], start=True, stop=True)
                        nc.tensor.matmul(ps[:nn, 512:1024], lhsT=lhsT,
                                         rhs=w2s[:, 0, 512:1024], start=True,
                                         stop=True)
                        nc.tensor.matmul(psc[:nn, 0:16], lhsT=lhsT,
                                         rhs=w2s[:, 0, 1024:PCOLS], start=True,
                                         stop=True)
                        row_cast(ps, psc, row, b * 4 + s)
                nc.sync.dma_start(
                    out=t2loc[:].rearrange("(a p) c -> p a c", p=P),
                    in_=row[:])
                psis2 = edges_prep(t2loc)
                nc.gpsimd.collective_compute(
                    "AllGather", mybir.AluOpType.bypass,
                    ins=[t2loc[0:nloc, :]], outs=[table2[:]],
                    replica_groups=[list(range(N_CORES))])
                return psis2

            # ---------------- edge phase ----------------
            def edges_prep(tloc):
                """Own-row gather + all si matmuls; runs in the AG window."""
                gown = spool.tile([P, T, ROWB], dt.float8e4, tag="gown")
                nc.gpsimd.dma_gather(gown[:], tloc[:], ownI[:], NPADC, NPADC,
                                     ROWB, single_packet=False, queue_num=0)
                psis = spool.tile([P, T, 160], dt.float32, tag="psis")
                for t in range(T):
                    hc_n = HC[t]
                    hc0 = sum(HC[:t])
                    sil = gown[:, t, HD + 16:HD + 24]          # [128, 8] fp8
                    psi = psip.tile([P, hc_n * 8], dt.float32, tag="psi")
                    for hc in range(hc_n):
                        nc.tensor.matmul(
                            psi[:, hc * 8:(hc + 1) * 8],
                            lhsT=ohtsb[:, hc0 + hc, :], rhs=sil,
                            start=True, stop=True)
                    nc.vector.tensor_copy(psis[:, t, 0:hc_n * 8], psi[:])
                return psis

            def edges(table, psis, layer):
                gps = None
                ebuf = None
                if layer == 2:
                    gps = psg.tile([1, HID], dt.float32, tag="gsum", name="gps")
                    ebuf = cpool.tile([P, T * HID], dt.bfloat16, name="ebuf")

                for t in range(T):
                    hc_n = HC[t]
                    c2_n = hc_n // 2
                    tail = hc_n % 2
                    hc0 = sum(HC[:t])
                    G = wpool.tile([P, hc_n, ROWB], dt.float8e4, tag="G")
                    h1 = (hc_n // 2) * P
                    nc.gpsimd.dma_gather(
                        G[:, 0:hc_n // 2, :], table[:],
                        srcI[:, hc0 * 8:(hc0 + hc_n // 2) * 8],
                        h1, h1, ROWB, single_packet=False, queue_num=0)
                    nc.gpsimd.dma_gather(
                        G[:, hc_n // 2:hc_n, :], table[:],
                        srcI[:, (hc0 + hc_n // 2) * 8:(hc0 + hc_n) * 8],
                        hc_n * P - h1, hc_n * P - h1, ROWB,
                        single_packet=False, queue_num=1)

                    # logits -> Exp(. - ln8)
                    LG = wpool.tile([P, hc_n, 8], dt.float32, tag="LG")
                    nc.vector.tensor_tensor(
                        out=LG[:],
                        in0=G[:, :, HD:HD + 16].bitcast(dt.bfloat16),
                        in1=psis[:, t, 0:hc_n * 8].rearrange(
                            "p (c e) -> p c e", e=8),
                        op=Alu.add)
                    LK = wpool.tile([P, hc_n, 8], dt.float32, tag="LK")
                    nc.vector.scalar_tensor_tensor(
                        out=LK[:], in0=LG[:], scalar=0.01, in1=LG[:],
                        op0=Alu.mult, op1=Alu.max)
                    EXf8 = wpool.tile([P, hc_n, 16], dt.float8e4, tag="EXf8")
                    nc.scalar.activation(EXf8[:, :, 0:8], LK[:], Act.Exp,
                                         bias=negln8[:])

                    ps = pspool.tile([P, HD], dt.float32, tag="ps")
                    psd = psip.tile([P, 16], dt.float32, tag="psc")
                    for c2 in range(c2_n):
                        g0 = 2 * c2
                        msg = kpool.tile([P, 2, HD], dt.float8e4, tag="msg")
                        nc.vector.tensor_tensor(
                            out=msg[:].rearrange("p i (h d) -> p i h d", h=HEADS),
                            in0=G[:, g0:g0 + 2, 0:HD].rearrange(
                                "p i (h d) -> p i h d", h=HEADS),
                            in1=EXf8[:, g0:g0 + 2, 0:8].unsqueeze(3).to_broadcast(
                                [P, 2, HEADS, HID]),
                            op=Alu.mult)
                        oh2 = ohsb[:, hc0 + g0:hc0 + g0 + 2, :]
                        st, sp = (c2 == 0), (c2 == c2_n - 1 and not tail)
                        nc.tensor.matmul(ps[:, 0:512], lhsT=oh2,
                                         rhs=msg[:, :, 0:512], start=st, stop=sp,
                                         perf_mode=DR)
                        nc.tensor.matmul(ps[:, 512:1024], lhsT=oh2,
                                         rhs=msg[:, :, 512:1024], start=st,
                                         stop=sp, perf_mode=DR)
                        nc.tensor.matmul(psd[:, 0:8], lhsT=oh2,
                                         rhs=EXf8[:, g0:g0 + 2, 0:8], start=st,
                                         stop=sp, perf_mode=DR)
                    if tail:
                        g0 = hc_n - 1
                        msg = kpool.tile([P, 2, HD], dt.float8e4, tag="msg",
                                         name="msgt")
                        nc.vector.tensor_tensor(
                            out=msg[:, 0:1, :].rearrange(
                                "p i (h d) -> p i h d", h=HEADS),
                            in0=G[:, g0:g0 + 1, 0:HD].rearrange(
                                "p i (h d) -> p i h d", h=HEADS),
                            in1=EXf8[:, g0:g0 + 1, 0:8].unsqueeze(3).to_broadcast(
                                [P, 1, HEADS, HID]),
                            op=Alu.mult)
                        oh1 = ohsb[:, hc0 + g0, :]
                        st = (c2_n == 0)
                        nc.tensor.matmul(ps[:, 0:512], lhsT=oh1,
                                         rhs=msg[:, 0, 0:512], start=st,
                                         stop=True)
                        nc.tensor.matmul(ps[:, 512:1024], lhsT=oh1,
                                         rhs=msg[:, 0, 512:1024], start=st,
                                         stop=True)
                        nc.tensor.matmul(psd[:, 0:8], lhsT=oh1,
                                         rhs=EXf8[:, g0, 0:8], start=st,
                                         stop=True)

                    # ---- postprocess tile ----
                    den = wpool.tile([P, 8], dt.float32, tag="den")
                    nc.vector.tensor_scalar(out=den[:], in0=psd[:, 0:8],
                                            scalar1=float(HEADS), scalar2=1e-30,
                                            op0=Alu.mult, op1=Alu.max)
                    rec = wpool.tile([P, 8], dt.float32, tag="rec")
                    nc.vector.reciprocal(rec[:], den[:])
                    wsum = spool.tile([P, HID, HEADS], dt.float32, tag="wsum")
                    for h in range(HEADS):
                        nc.scalar.activation(
                            wsum[:, :, h], ps[:, h * HID:(h + 1) * HID],
                            Act.Copy, scale=rec[:, h:h + 1])
                    S = wpool.tile([P, HID], dt.float32, tag="S")
                    nc.vector.tensor_reduce(S[:].unsqueeze(2), wsum[:],
                                            axis=mybir.AxisListType.X,
                                            op=Alu.add)
                    # elu(S) = exp(min(S,0)) - 1 + max(S,0)
                    neg = wpool.tile([P, HID], dt.float32, tag="neg")
                    nc.vector.tensor_tensor(out=neg[:], in0=S[:], in1=zeros[:],
                                            op=Alu.min)
                    en = wpool.tile([P, HID], dt.float32, tag="en")
                    nc.scalar.activation(en[:], neg[:], Act.Exp)
                    pos = wpool.tile([P, HID], dt.float32, tag="pos")
                    nc.vector.tensor_tensor(out=pos[:], in0=S[:], in1=zeros[:],
                                            op=Alu.max)
                    if layer == 1:
                        ebf = wpool.tile([P, HID], dt.bfloat16, tag="ebf")
                        nc.vector.scalar_tensor_tensor(
                            out=ebf[:], in0=en[:], scalar=-1.0, in1=pos[:],
                            op0=Alu.add, op1=Alu.add)
                        nn_t = min(P, nloc - t * P)
                        nc.sync.dma_start(out=emb_loc[t * P:t * P + nn_t, :],
                                          in_=ebf[:nn_t, :])
                    else:
                        nc.vector.scalar_tensor_tensor(
                            out=ebuf[:, t * HID:(t + 1) * HID], in0=en[:],
                            scalar=-1.0, in1=pos[:], op0=Alu.add, op1=Alu.add)
                if layer == 2:
                    for t in range(T):
                        nn_t = min(P, nloc - t * P)
                        nc.tensor.matmul(gps[0:1, :], lhsT=ones_col[:nn_t, :],
                                         rhs=ebuf[:nn_t, t * HID:(t + 1) * HID],
                                         start=(t == 0), stop=(t == T - 1))
                return gps

            def zero_out_vec():
                z = kpool.tile([1, HID], dt.float32, tag="gout", name="z")
                nc.gpsimd.memset(z[:], 0.0)
                nc.sync.dma_start(out=out_vec[:], in_=z[:])

            def flow():
                psis1 = proj1()
                if debug:
                    tb = wpool.tile([P, ROWB], dt.float8e4, tag="rowA", name="tb")
                    nc.sync.dma_start(out=tb[:], in_=table1[0:P, :])
                    tbf = wpool.tile([P, ROWB], dt.float32, tag="tbf")
                    nc.vector.tensor_copy(tbf[:], tb[:])
                    nc.sync.dma_start(out=dbg["tab1"][:], in_=tbf[:])
                if stages >= 2:
                    edges(table1, psis1, layer=1)
                if stages >= 4:
                    psis2 = proj2()
                if stages >= 5:
                    gps = edges(table2, psis2, layer=2)
                    gout = wpool.tile([1, HID], dt.float32, tag="gout")
                    nc.vector.tensor_copy(gout[:], gps[:])
                    nc.sync.dma_start(out=out_vec[:], in_=gout[:])
                else:
                    zero_out_vec()

            for _it in range(iters):
                flow()

    nc.compile()
    return nc


# ----------------------------------------------------------------------------
# top-level kernel
# ----------------------------------------------------------------------------

_CACHE = {}


def host_finish(partials, ln_g, ln_b, Wl1, bl1, Wl2, bl2, Wl3, bl3):
    g = partials.sum(axis=0) / np.float64(N_NODES)
    mu = g.mean()
    var = ((g - mu) ** 2).mean()
    gn = (g - mu) / np.sqrt(var + 1e-5) * ln_g + ln_b
    x = Wl1 @ gn + bl1
    x = np.maximum(x, 0.01 * x)
    x = Wl2 @ x + bl2
    x = np.maximum(x, 0.01 * x)
    x = Wl3 @ x + bl3
    return np.maximum(x, 0.0).astype(np.float32)


def kernel(node_features, edge_src, edge_dst, W1, a1, W2, a2,
           ln_g, ln_b, Wl1, bl1, Wl2, bl2, Wl3, bl3):
    from concourse.bass_utils import run_bass_kernel_spmd
    node_features = np.asarray(node_features, dtype=np.float32)
    edge_src = np.asarray(edge_src, dtype=np.int32)
    edge_dst = np.asarray(edge_dst, dtype=np.int32)
    in_maps, meta = host_prep(node_features, edge_src, edge_dst,
                              np.asarray(W1, np.float32), np.asarray(a1, np.float32),
                              np.asarray(W2, np.float32), np.asarray(a2, np.float32))
    if "prog" not in _CACHE:
        _CACHE["prog"] = build_program(meta)
    res = run_bass_kernel_spmd(_CACHE["prog"], in_maps,
                               core_ids=list(range(N_CORES)))
    partials = np.stack([res.results[c]["out_vec"][0] for c in range(N_CORES)])
    return host_finish(partials.astype(np.float64),
                       np.asarray(ln_g, np.float64), np.asarray(ln_b, np.float64),
                       np.asarray(Wl1, np.float64), np.asarray(bl1, np.float64),
                       np.asarray(Wl2, np.float64), np.asarray(bl2, np.float64),
                       np.asarray(Wl3, np.float64), np.asarray(bl3, np.float64))


# revision 28
# speedup vs baseline: 1.0989x; 1.0989x over previous
"""GAT (2-layer, 8-head) Trainium2 Bass kernel, SPMD over 8 NeuronCores. v2.

Strategy (dst-node-parallel, fp8 tables, host-precomputed onehots):
  - Host: sort edges by dst; each core owns a 1250-node dst range (10 tiles of
    128). Edges per tile padded to 256-multiples (fp8 DoubleRow chunks).
    Host precomputes: fused projection weights, per-slot onehot matrices
    oh[p,hc,j] (edge->dst-local) and their transposes ohT[j,hc,p] in fp8, and
    wrapped int16 gather indices.
  - Table rows are 1280B fp8: [h fp8 1024B | sj bf16 16B | si fp8 8B | pad].
  - Layer-1 projection is SHARDED: each core projects only its 1250 nodes
    (bf16 matmuls for precision), then AllGather assembles the full fp8 table.
  - Edge phase per tile: dma_gather rows by src; si of the 128 owned dst nodes
    comes from a tiny own-row gather + per-halfchunk onehot-transpose matmuls
    (no per-edge si gather). Logits -> Lrelu -> Exp(-ln8) on ACT. Messages
    alpha*h via ONE broadcast tensor_tensor per 256-edge chunk (fp8 out).
    Aggregation via fp8 DoubleRow onehot-matmuls into PSUM (denominator =
    extra matmul columns). Head-mean via mult+reduce, ELU -> emb tile.
  - AllGather emb1 (bf16), replicated bf16 layer-2 projection, second edge
    phase -> per-core partial graph-sum [1, 128].
  - Host: sum partials, LayerNorm + MLP head -> [16].
"""

import numpy as np
import ml_dtypes

BF16 = ml_dtypes.bfloat16

N_NODES = 10000
N_EDGES = 160000
N_FEAT = 512
HEADS = 8
HID = 128
OUT = 16

N_CORES = 8
P = 128

nloc = N_NODES // N_CORES          # 1250
NPADC = 1280                       # per-core padded node rows
T = NPADC // P                     # 10 tiles per core
HD = HEADS * HID                   # 1024
PCOLS = HD + 16                    # proj psum cols: [h | sj 8 | si 8]
ROWB = 1280                        # table row bytes (fp8 dtype)
LN8 = float(np.log(8.0))


def _wrap_idx(idx: np.ndarray) -> np.ndarray:
    """[n] -> [128, n//16] wrapped int16 layout for dma_gather."""
    n = idx.shape[0]
    assert n % 16 == 0
    w = idx.astype(np.int16).reshape(n // 16, 16).T
    return np.tile(w, (8, 1))


def host_prep(node_features, edge_src, edge_dst, W1, a1, W2, a2):
    F8 = ml_dtypes.float8_e4m3

    order = np.argsort(edge_dst, kind="stable")
    src_s = edge_src[order].astype(np.int64)
    dst_s = edge_dst[order].astype(np.int64)

    # per (core, tile) edge counts -> common 256-chunk counts per tile pos
    cnt = np.zeros((N_CORES, T), dtype=np.int64)
    bounds = {}
    for c in range(N_CORES):
        base = c * nloc
        for t in range(T):
            n0 = base + t * P
            n1 = min(base + (t + 1) * P, (c + 1) * nloc)
            lo = np.searchsorted(dst_s, n0, side="left")
            hi = np.searchsorted(dst_s, n1, side="left")
            cnt[c, t] = hi - lo
            bounds[(c, t)] = (lo, hi)
    HC = [int(-(-cnt[:, t].max() // 128)) for t in range(T)]  # 128-halfchunks
    tot_hc = sum(HC)
    tot_slots = tot_hc * P

    # fused projection matrices [K, 1040] = [h | sj | si]
    def wcat(W, a, K):
        w = np.transpose(W, (2, 0, 1)).reshape(K, HD)
        si = np.einsum("hdf,hd->fh", W, a[:, :HID])   # dst-side
        sj = np.einsum("hdf,hd->fh", W, a[:, HID:])   # src-side
        return np.concatenate([w, sj, si], axis=1).astype(BF16)

    w1c = wcat(W1, a1, N_FEAT)   # [512, 1040]
    w2c = wcat(W2, a2, HID)      # [128, 1040]
    # sbuf layouts: w1s[p, k, col] = w1c[k*128+p, col]
    w1s = np.ascontiguousarray(
        w1c.reshape(4, P, PCOLS).transpose(1, 0, 2))      # [128, 4, 1040]
    w2s = np.ascontiguousarray(w2c.reshape(1, P, PCOLS).transpose(1, 0, 2))

    xbf = node_features.astype(BF16)

    in_maps = []
    for c in range(N_CORES):
        base = c * nloc
        src_pad = np.zeros(tot_slots, dtype=np.int64)
        dstl = np.full(tot_slots, -1, dtype=np.int64)  # -1 -> pad slot
        off = 0
        for t in range(T):
            lo, hi = bounds[(c, t)]
            k = hi - lo
            src_pad[off:off + k] = src_s[lo:hi]
            dstl[off:off + k] = dst_s[lo:hi] - (base + t * P)
            off += HC[t] * P
        assert off == tot_slots

        # onehots fp8: oh[p, hc, j], ohT[j, hc, p]
        sl = src_pad.reshape(tot_hc, P)       # slot (hc, p)
        dl = dstl.reshape(tot_hc, P)
        jj = np.arange(P)
        oh = (dl[:, :, None] == jj[None, None, :])        # [hc, p, j]
        oh8 = np.ascontiguousarray(
            oh.transpose(1, 0, 2).astype(F8).reshape(P, tot_hc * P))
        ohT8 = np.ascontiguousarray(
            oh.transpose(2, 0, 1).astype(F8).reshape(P, tot_hc * P))

        srcI = np.concatenate(
            [_wrap_idx(sl[sum(HC[:t]):sum(HC[:t + 1])].reshape(-1))
             for t in range(T)], axis=1)                  # [128, tot_slots/16]
        own = np.minimum(np.arange(NPADC), nloc - 1)      # local row ids
        own16 = _wrap_idx(own)                            # [128, 80]

        # per-core x slice, transposed: xT[p, k, n] = x[base+n, k*128+p]
        xsl = np.zeros((NPADC, N_FEAT), dtype=BF16)
        xsl[:nloc] = xbf[base:base + nloc]
        xT = np.ascontiguousarray(
            xsl.reshape(NPADC, 4, P).transpose(2, 1, 0))  # [128, 4, 1280]

        in_maps.append({
            "xT": xT, "w1s": w1s, "w2s": w2s,
            "srcI": np.ascontiguousarray(srcI), "own16": own16,
            "oh8": oh8, "ohT8": ohT8,
        })

    meta = {"HC": HC, "tot_hc": tot_hc}
    return in_maps, meta


def build_program(meta, debug=False, stages=5, iters=1):
    import concourse.bacc as bacc
    import concourse.mybir as mybir
    import concourse.tile as tile
    from concourse.library_config import mlp

    dt = mybir.dt
    Alu = mybir.AluOpType
    Act = mybir.ActivationFunctionType
    DR = mybir.MatmulPerfMode.DoubleRow

    HC = meta["HC"]
    tot_hc = meta["tot_hc"]

    nc = bacc.Bacc("TRN2", num_devices=N_CORES, num_swdge_queues=2,
                   dynamic_dma_scratch_size=49152)

    xT_d = nc.dram_tensor("xT", [P, 4, NPADC], dt.bfloat16, kind="ExternalInput")
    w1_d = nc.dram_tensor("w1s", [P, 4, PCOLS], dt.bfloat16, kind="ExternalInput")
    w2_d = nc.dram_tensor("w2s", [P, 1, PCOLS], dt.bfloat16, kind="ExternalInput")
    srcI_d = nc.dram_tensor("srcI", [P, tot_hc * 8], dt.int16, kind="ExternalInput")
    own_d = nc.dram_tensor("own16", [P, NPADC // 16], dt.int16, kind="ExternalInput")
    oh_d = nc.dram_tensor("oh8", [P, tot_hc * P], dt.float8e4, kind="ExternalInput")
    ohT_d = nc.dram_tensor("ohT8", [P, tot_hc * P], dt.float8e4, kind="ExternalInput")

    out_vec = nc.dram_tensor("out_vec", [1, HID], dt.float32, kind="ExternalOutput")
    dbg = {}
    if debug:
        dbg["emb1"] = nc.dram_tensor("dbg_emb1", [N_NODES, HID], dt.float32,
                                     kind="ExternalOutput")
        dbg["tab1"] = nc.dram_tensor("dbg_tab1", [P, ROWB], dt.float32,
                                     kind="ExternalOutput")

    t1loc = nc.dram_tensor("t1loc", [NPADC, ROWB], dt.float8e4)
    table1 = nc.dram_tensor("table1", [N_NODES, ROWB], dt.float8e4,
                            addr_space="Shared")
    t2loc = nc.dram_tensor("t2loc", [NPADC, ROWB], dt.float8e4)
    table2 = nc.dram_tensor("table2", [N_NODES, ROWB], dt.float8e4,
                            addr_space="Shared")
    emb_loc = nc.dram_tensor("emb_loc", [NPADC, HID], dt.bfloat16)

    with tile.TileContext(nc) as tc:
        with (
            tc.tile_pool(name="const", bufs=1) as cpool,
            tc.tile_pool(name="xload", bufs=2) as xpool,
            tc.tile_pool(name="single", bufs=1) as spool,
            tc.tile_pool(name="work", bufs=2) as wpool,
            tc.tile_pool(name="chunk", bufs=3) as kpool,
            tc.tile_pool(name="psum", bufs=2, space="PSUM") as pspool,
            tc.tile_pool(name="psi", bufs=1, space="PSUM") as psip,
            tc.tile_pool(name="psg", bufs=1, space="PSUM") as psg,
        ):
            nc.gpsimd.load_library(mlp)

            ones_col = cpool.tile([P, 1], dt.bfloat16)
            nc.gpsimd.memset(ones_col[:], 1.0)
            negln8 = cpool.tile([P, 1], dt.float32)
            nc.gpsimd.memset(negln8[:], -LN8)
            zeros = cpool.tile([P, HID], dt.float32)
            nc.gpsimd.memset(zeros[:], 0.0)

            w1s = cpool.tile([P, 4, PCOLS], dt.bfloat16)
            nc.sync.dma_start(out=w1s[:], in_=w1_d[:])
            w2s = cpool.tile([P, 1, PCOLS], dt.bfloat16)
            nc.sync.dma_start(out=w2s[:], in_=w2_d[:])
            srcI = cpool.tile([P, tot_hc * 8], dt.int16)
            nc.sync.dma_start(out=srcI[:], in_=srcI_d[:])
            ownI = cpool.tile([P, NPADC // 16], dt.int16)
            nc.sync.dma_start(out=ownI[:], in_=own_d[:])
            ohsb = cpool.tile([P, tot_hc, P], dt.float8e4)
            nc.scalar.dma_start(out=ohsb[:],
                                in_=oh_d[:].rearrange("p (c j) -> p c j", j=P))
            ohtsb = cpool.tile([P, tot_hc, P], dt.float8e4)
            nc.scalar.dma_start(out=ohtsb[:],
                                in_=ohT_d[:].rearrange("p (c j) -> p c j", j=P))

            # ---------------- row cast helper ----------------
            def row_cast(ps, psc, row, s):
                """PSUM h[*,1024]+scores[*,16] f32 -> fp8 row at row[:,s,:]."""
                nc.scalar.activation(row[:, s, 0:512], ps[:, 0:512], Act.Copy)
                nc.vector.tensor_copy(row[:, s, 512:HD], ps[:, 512:HD])
                nc.vector.tensor_copy(
                    row[:, s, HD:HD + 16].bitcast(dt.bfloat16), psc[:, 0:8])
                nc.vector.tensor_copy(row[:, s, HD + 16:HD + 24], psc[:, 8:16])

            # ---------------- layer-1 sharded projection ----------------
            def proj1():
                xsb = spool.tile([P, 4, NPADC], dt.bfloat16, tag="xsb")
                nc.sync.dma_start(out=xsb[:], in_=xT_d[:])
                row = spool.tile([P, T, ROWB], dt.float8e4, tag="rowA")
                for s in range(T):
                    ps = pspool.tile([P, HD], dt.float32, tag="ps")
                    psc = psip.tile([P, 16], dt.float32, tag="psc")
                    for k in range(4):
                        lhsT = xsb[:, k, s * P:(s + 1) * P]
                        st, sp = (k == 0), (k == 3)
                        nc.tensor.matmul(ps[:, 0:512], lhsT=lhsT,
                                         rhs=w1s[:, k, 0:512], start=st, stop=sp)
                        nc.tensor.matmul(ps[:, 512:1024], lhsT=lhsT,
                                         rhs=w1s[:, k, 512:1024], start=st, stop=sp)
                        nc.tensor.matmul(psc[:, 0:16], lhsT=lhsT,
                                         rhs=w1s[:, k, 1024:PCOLS], start=st, stop=sp)
                    row_cast(ps, psc, row, s)
                nc.sync.dma_start(
                    out=t1loc[:].rearrange("(a p) c -> p a c", p=P),
                    in_=row[:])
                psis1 = edges_prep(t1loc)
                nc.gpsimd.collective_compute(
                    "AllGather", mybir.AluOpType.bypass,
                    ins=[t1loc[0:nloc, :]], outs=[table1[:]],
                    replica_groups=[list(range(N_CORES))])
                return psis1

            # ---------------- layer-2 sharded projection ----------------
            def proj2():
                row = spool.tile([P, T, ROWB], dt.float8e4, tag="rowA",
                                 name="row2")
                for b in range(3):
                    r0 = b * 512
                    rn = min(512, NPADC - r0)         # 512, 512, 256
                    eT = xpool.tile([P, 512], dt.bfloat16, tag="eT")
                    eng = nc.sync if b % 2 == 0 else nc.scalar
                    eng.dma_start_transpose(eT[:, :rn], emb_loc[r0:r0 + rn, :])
                    for s in range(-(-rn // P)):
                        nn = min(P, rn - s * P)
                        ps = pspool.tile([P, HD], dt.float32, tag="ps")
                        psc = psip.tile([P, 16], dt.float32, tag="psc")
                        lhsT = eT[:, s * P:s * P + nn]
                        nc.tensor.matmul(ps[:nn, 0:512], lhsT=lhsT,
                                         rhs=w2s[:, 0, 0:512], start=True, stop=True)
                        nc.tensor.matmul(ps[:nn, 512:1024], lhsT=lhsT,
                                         rhs=w2s[:, 0, 512:1024], start=True,
                                         stop=True)
                        nc.tensor.matmul(psc[:nn, 0:16], lhsT=lhsT,
                                         rhs=w2s[:, 0, 1024:PCOLS], start=True,
                                         stop=True)
                        row_cast(ps, psc, row, b * 4 + s)
                nc.sync.dma_start(
                    out=t2loc[:].rearrange("(a p) c -> p a c", p=P),
                    in_=row[:])
                psis2 = edges_prep(t2loc)
                nc.gpsimd.collective_compute(
                    "AllGather", mybir.AluOpType.bypass,
                    ins=[t2loc[0:nloc, :]], outs=[table2[:]],
                    replica_groups=[list(range(N_CORES))])
                return psis2

            # ---------------- edge phase ----------------
            def edges_prep(tloc):
                """Own-row gather + all si matmuls; runs in the AG window."""
                gown = spool.tile([P, T, ROWB], dt.float8e4, tag="gown")
                nc.gpsimd.dma_gather(gown[:], tloc[:], ownI[:], NPADC, NPADC,
                                     ROWB, single_packet=False, queue_num=0)
                psis = spool.tile([P, T, 160], dt.float32, tag="psis")
                for t in range(T):
                    hc_n = HC[t]
                    hc0 = sum(HC[:t])
                    sil = gown[:, t, HD + 16:HD + 24]          # [128, 8] fp8
                    psi = psip.tile([P, hc_n * 8], dt.float32, tag="psi")
                    for hc in range(hc_n):
                        nc.tensor.matmul(
                            psi[:, hc * 8:(hc + 1) * 8],
                            lhsT=ohtsb[:, hc0 + hc, :], rhs=sil,
                            start=True, stop=True)
                    nc.vector.tensor_copy(psis[:, t, 0:hc_n * 8], psi[:])
                return psis

            def edges(table, psis, layer):
                gps = None
                ebuf = None
                if layer == 2:
                    gps = psg.tile([1, HID], dt.float32, tag="gsum", name="gps")
                    ebuf = cpool.tile([P, T * HID], dt.bfloat16, name="ebuf")

                for t in range(T):
                    hc_n = HC[t]
                    c2_n = hc_n // 2
                    tail = hc_n % 2
                    hc0 = sum(HC[:t])
                    G = wpool.tile([P, hc_n, ROWB], dt.float8e4, tag="G")
                    h1 = (hc_n // 2) * P
                    nc.gpsimd.dma_gather(
                        G[:, 0:hc_n // 2, :], table[:],
                        srcI[:, hc0 * 8:(hc0 + hc_n // 2) * 8],
                        h1, h1, ROWB, single_packet=False, queue_num=0)
                    nc.gpsimd.dma_gather(
                        G[:, hc_n // 2:hc_n, :], table[:],
                        srcI[:, (hc0 + hc_n // 2) * 8:(hc0 + hc_n) * 8],
                        hc_n * P - h1, hc_n * P - h1, ROWB,
                        single_packet=False, queue_num=1)

                    # logits -> Exp(. - ln8)
                    LG = wpool.tile([P, hc_n, 8], dt.float32, tag="LG")
                    nc.vector.tensor_tensor(
                        out=LG[:],
                        in0=G[:, :, HD:HD + 16].bitcast(dt.bfloat16),
                        in1=psis[:, t, 0:hc_n * 8].rearrange(
                            "p (c e) -> p c e", e=8),
                        op=Alu.add)
                    LK = wpool.tile([P, hc_n, 8], dt.float32, tag="LK")
                    nc.vector.scalar_tensor_tensor(
                        out=LK[:], in0=LG[:], scalar=0.01, in1=LG[:],
                        op0=Alu.mult, op1=Alu.max)
                    EXf8 = wpool.tile([P, hc_n, 16], dt.float8e4, tag="EXf8")
                    nc.scalar.activation(EXf8[:, :, 0:8], LK[:], Act.Exp,
                                         bias=negln8[:])
                    EXf32 = wpool.tile([P, hc_n, 8], dt.float32, tag="EXf32")
                    nc.scalar.activation(EXf32[:], LK[:], Act.Exp,
                                         bias=negln8[:])

                    ps = pspool.tile([P, HD], dt.float32, tag="ps")
                    psd = psip.tile([P, 16], dt.float32, tag="psc")
                    for c2 in range(c2_n):
                        g0 = 2 * c2
                        msg = kpool.tile([P, 2, HD], dt.float8e4, tag="msg")
                        nc.vector.tensor_tensor(
                            out=msg[:, :, 0:768].rearrange(
                                "p i (h d) -> p i h d", h=6),
                            in0=G[:, g0:g0 + 2, 0:768].rearrange(
                                "p i (h d) -> p i h d", h=6),
                            in1=EXf8[:, g0:g0 + 2, 0:6].unsqueeze(3).to_broadcast(
                                [P, 2, 6, HID]),
                            op=Alu.mult)
                        for i in range(2):
                            for h in (6, 7):
                                nc.scalar.activation(
                                    msg[:, i, h * HID:(h + 1) * HID],
                                    G[:, g0 + i, h * HID:(h + 1) * HID],
                                    Act.Copy,
                                    scale=EXf32[:, g0 + i, h:h + 1])
                        oh2 = ohsb[:, hc0 + g0:hc0 + g0 + 2, :]
                        st, sp = (c2 == 0), (c2 == c2_n - 1 and not tail)
                        nc.tensor.matmul(ps[:, 0:512], lhsT=oh2,
                                         rhs=msg[:, :, 0:512], start=st, stop=sp,
                                         perf_mode=DR)
                        nc.tensor.matmul(ps[:, 512:1024], lhsT=oh2,
                                         rhs=msg[:, :, 512:1024], start=st,
                                         stop=sp, perf_mode=DR)
                        nc.tensor.matmul(psd[:, 0:8], lhsT=oh2,
                                         rhs=EXf8[:, g0:g0 + 2, 0:8], start=st,
                                         stop=sp, perf_mode=DR)
                    if tail:
                        g0 = hc_n - 1
                        msg = kpool.tile([P, 2, HD], dt.float8e4, tag="msg",
                                         name="msgt")
                        nc.vector.tensor_tensor(
                            out=msg[:, 0:1, 0:768].rearrange(
                                "p i (h d) -> p i h d", h=6),
                            in0=G[:, g0:g0 + 1, 0:768].rearrange(
                                "p i (h d) -> p i h d", h=6),
                            in1=EXf8[:, g0:g0 + 1, 0:6].unsqueeze(3).to_broadcast(
                                [P, 1, 6, HID]),
                            op=Alu.mult)
                        for h in (6, 7):
                            nc.scalar.activation(
                                msg[:, 0, h * HID:(h + 1) * HID],
                                G[:, g0, h * HID:(h + 1) * HID],
                                Act.Copy,
                                scale=EXf32[:, g0, h:h + 1])
                        oh1 = ohsb[:, hc0 + g0, :]
                        st = (c2_n == 0)
                        nc.tensor.matmul(ps[:, 0:512], lhsT=oh1,
                                         rhs=msg[:, 0, 0:512], start=st,
                                         stop=True)
                        nc.tensor.matmul(ps[:, 512:1024], lhsT=oh1,
                                         rhs=msg[:, 0, 512:1024], start=st,
                                         stop=True)
                        nc.tensor.matmul(psd[:, 0:8], lhsT=oh1,
                                         rhs=EXf8[:, g0, 0:8], start=st,
                                         stop=True)

                    # ---- postprocess tile ----
                    den = wpool.tile([P, 8], dt.float32, tag="den")
                    nc.vector.tensor_scalar(out=den[:], in0=psd[:, 0:8],
                                            scalar1=float(HEADS), scalar2=1e-30,
                                            op0=Alu.mult, op1=Alu.max)
                    rec = wpool.tile([P, 8], dt.float32, tag="rec")
                    nc.vector.reciprocal(rec[:], den[:])
                    wsum = spool.tile([P, HID, HEADS], dt.float32, tag="wsum")
                    nc.vector.tensor_tensor(
                        out=wsum[:],
                        in0=ps[:, 0:HD].rearrange("p (h d) -> p d h", h=HEADS),
                        in1=rec[:].unsqueeze(1).to_broadcast([P, HID, HEADS]),
                        op=Alu.mult)
                    S = wpool.tile([P, HID], dt.float32, tag="S")
                    nc.vector.tensor_reduce(S[:].unsqueeze(2), wsum[:],
                                            axis=mybir.AxisListType.X,
                                            op=Alu.add)
                    # elu(S) = exp(min(S,0)) - 1 + max(S,0)
                    neg = wpool.tile([P, HID], dt.float32, tag="neg")
                    nc.vector.tensor_tensor(out=neg[:], in0=S[:], in1=zeros[:],
                                            op=Alu.min)
                    en = wpool.tile([P, HID], dt.float32, tag="en")
                    nc.scalar.activation(en[:], neg[:], Act.Exp)
                    pos = wpool.tile([P, HID], dt.float32, tag="pos")
                    nc.vector.tensor_tensor(out=pos[:], in0=S[:], in1=zeros[:],
                                            op=Alu.max)
                    if layer == 1:
                        ebf = wpool.tile([P, HID], dt.bfloat16, tag="ebf")
                        nc.vector.scalar_tensor_tensor(
                            out=ebf[:], in0=en[:], scalar=-1.0, in1=pos[:],
                            op0=Alu.add, op1=Alu.add)
                        nn_t = min(P, nloc - t * P)
                        nc.sync.dma_start(out=emb_loc[t * P:t * P + nn_t, :],
                                          in_=ebf[:nn_t, :])
                    else:
                        nc.vector.scalar_tensor_tensor(
                            out=ebuf[:, t * HID:(t + 1) * HID], in0=en[:],
                            scalar=-1.0, in1=pos[:], op0=Alu.add, op1=Alu.add)
                if layer == 2:
                    for t in range(T):
                        nn_t = min(P, nloc - t * P)
                        nc.tensor.matmul(gps[0:1, :], lhsT=ones_col[:nn_t, :],
                                         rhs=ebuf[:nn_t, t * HID:(t + 1) * HID],
                                         start=(t == 0), stop=(t == T - 1))
                return gps

            def zero_out_vec():
                z = kpool.tile([1, HID], dt.float32, tag="gout", name="z")
                nc.gpsimd.memset(z[:], 0.0)
                nc.sync.dma_start(out=out_vec[:], in_=z[:])

            def flow():
                psis1 = proj1()
                if debug:
                    tb = wpool.tile([P, ROWB], dt.float8e4, tag="rowA", name="tb")
                    nc.sync.dma_start(out=tb[:], in_=table1[0:P, :])
                    tbf = wpool.tile([P, ROWB], dt.float32, tag="tbf")
                    nc.vector.tensor_copy(tbf[:], tb[:])
                    nc.sync.dma_start(out=dbg["tab1"][:], in_=tbf[:])
                if stages >= 2:
                    edges(table1, psis1, layer=1)
                if stages >= 4:
                    psis2 = proj2()
                if stages >= 5:
                    gps = edges(table2, psis2, layer=2)
                    gout = wpool.tile([1, HID], dt.float32, tag="gout")
                    nc.vector.tensor_copy(gout[:], gps[:])
                    nc.sync.dma_start(out=out_vec[:], in_=gout[:])
                else:
                    zero_out_vec()

            for _it in range(iters):
                flow()

    nc.compile()
    return nc


# ----------------------------------------------------------------------------
# top-level kernel
# ----------------------------------------------------------------------------

_CACHE = {}


def host_finish(partials, ln_g, ln_b, Wl1, bl1, Wl2, bl2, Wl3, bl3):
    g = partials.sum(axis=0) / np.float64(N_NODES)
    mu = g.mean()
    var = ((g - mu) ** 2).mean()
    gn = (g - mu) / np.sqrt(var + 1e-5) * ln_g + ln_b
    x = Wl1 @ gn + bl1
    x = np.maximum(x, 0.01 * x)
    x = Wl2 @ x + bl2
    x = np.maximum(x, 0.01 * x)
    x = Wl3 @ x + bl3
    return np.maximum(x, 0.0).astype(np.float32)


def kernel(node_features, edge_src, edge_dst, W1, a1, W2, a2,
           ln_g, ln_b, Wl1, bl1, Wl2, bl2, Wl3, bl3):
    from concourse.bass_utils import run_bass_kernel_spmd
    node_features = np.asarray(node_features, dtype=np.float32)
    edge_src = np.asarray(edge_src, dtype=np.int32)
    edge_dst = np.asarray(edge_dst, dtype=np.int32)
    in_maps, meta = host_prep(node_features, edge_src, edge_dst,
                              np.asarray(W1, np.float32), np.asarray(a1, np.float32),
                              np.asarray(W2, np.float32), np.asarray(a2, np.float32))
    if "prog" not in _CACHE:
        _CACHE["prog"] = build_program(meta)
    res = run_bass_kernel_spmd(_CACHE["prog"], in_maps,
                               core_ids=list(range(N_CORES)))
    partials = np.stack([res.results[c]["out_vec"][0] for c in range(N_CORES)])
    return host_finish(partials.astype(np.float64),
                       np.asarray(ln_g, np.float64), np.asarray(ln_b, np.float64),
                       np.asarray(Wl1, np.float64), np.asarray(bl1, np.float64),
                       np.asarray(Wl2, np.float64), np.asarray(bl2, np.float64),
                       np.asarray(Wl3, np.float64), np.asarray(bl3, np.float64))
